# revision 8
# baseline (speedup 1.0000x reference)
"""AdditiveAttention Trainium2 kernel.

Problem (hardcoded shapes): B=16, Nq=128, Nk=256, D=256, H=256, V=256, f32.
  q = queries @ W_q.T ; k = keys @ W_k.T
  scores[b,q,k] = sum_h w_v[h] * tanh(q[b,q,h] + k[b,k,h])
  masked softmax over k (k >= valid_len -> -1e6), out = attn @ values

Sharding: data-parallel, 2 batches per core across 8 cores.

Per-core device program (per batch):
  - PE: q_projT (h x q), k_projT (h x k) from host-pretransposed inputs
  - DVE: feature[h, (q,hc,k)] = k_projT + q_projT[:,q] (per-q tensor_scalar add)
  - ACT: tanh over 8192-wide chunks
  - PE: scoresT[k,q] = sum_h w_v[h]*T via stationary-T matmuls (moving = w_v, N=1)
  - ACT: exp(scoresT + mask_bias)  (no max subtraction: |scores| <= ||w_v||_1)
  - PE: out_unnorm = expT.T @ values, den = expT.T @ ones ; DVE: out = out_unnorm/den
valid_len==0 batches: host zeroes w_v and mask -> scores=0 -> exact uniform softmax,
matching jax softmax of an all -1e6 row.
"""

import numpy as np

B, NQ, NK, D, H, V = 16, 128, 256, 256, 256, 256
NCORES = 8
BPC = B // NCORES  # batches per core
NQC = 16           # q's per feature chunk
NCHUNK = NQ // NQC

_CACHE = {}


def _build_nc(reps=1, mm_dtype="f32"):
    import contextlib
    import concourse.bass as bass
    import concourse.tile as tile
    from concourse import bacc, mybir

    f32 = mybir.dt.float32
    AF = mybir.ActivationFunctionType
    # dtype of the tanh output + w_v used in the scores matmul
    t_dt = {"f32": f32, "f32r": f32, "bf16": mybir.dt.bfloat16}[mm_dtype]

    def mm_ap(ap):
        return ap.bitcast(mybir.dt.float32r) if mm_dtype == "f32r" else ap

    nc = bacc.Bacc("TRN2")
    qT_d = nc.dram_tensor("qT", (BPC, D, NQ), f32, kind="ExternalInput")
    kT_d = nc.dram_tensor("kT", (BPC, D, NK), f32, kind="ExternalInput")
    vals_d = nc.dram_tensor("vals", (BPC, NK, V), f32, kind="ExternalInput")
    WqT_d = nc.dram_tensor("WqT", (D, H), f32, kind="ExternalInput")
    WkT_d = nc.dram_tensor("WkT", (D, H), f32, kind="ExternalInput")
    wv_d = nc.dram_tensor("wv", (BPC, H, 1), f32, kind="ExternalInput")
    em_d = nc.dram_tensor("emask", (BPC, NK, 1), f32, kind="ExternalInput")
    ones_d = nc.dram_tensor("ones", (128, 1), f32, kind="ExternalInput")
    out_d = nc.dram_tensor("out", (BPC, NQ, V), f32, kind="ExternalOutput")

    FW = 2 * NQC * 256  # feature chunk free width (q-local, hc, k)

    with tile.TileContext(nc) as tc:
        rep_loop = tc.For_i(0, reps, 1) if reps != 1 else contextlib.nullcontext()
        with (
            rep_loop,
            tc.tile_pool(name="const", bufs=1) as constp,
            tc.tile_pool(name="inb", bufs=2) as inp,
            tc.tile_pool(name="proj", bufs=2) as projp,
            tc.tile_pool(name="feat", bufs=2) as featp,
            tc.tile_pool(name="tanh", bufs=2) as tanhp,
            tc.tile_pool(name="eps", bufs=2) as epsp,
            tc.tile_pool(name="outb", bufs=2) as outbp,
            tc.tile_pool(name="ps_proj", bufs=1, space=bass.MemorySpace.PSUM) as psproj,
            tc.tile_pool(name="ps_s", bufs=2, space=bass.MemorySpace.PSUM) as pss,
            tc.tile_pool(name="ps_o", bufs=2, space=bass.MemorySpace.PSUM) as pso,
        ):
            # ---- constants ----
            Wq_sb = constp.tile([128, 2 * H], f32, tag="Wq")  # [:, dt*256+h]
            Wk_sb = constp.tile([128, 2 * H], f32, tag="Wk")
            for dt in range(2):
                nc.sync.dma_start(
                    Wq_sb[:, dt * H:(dt + 1) * H], WqT_d[dt * 128:(dt + 1) * 128, :])
                nc.sync.dma_start(
                    Wk_sb[:, dt * H:(dt + 1) * H], WkT_d[dt * 128:(dt + 1) * 128, :])
            wv_f32 = constp.tile([128, 2 * BPC], f32, tag="wvf")  # col i*2+hc
            em_sb = constp.tile([128, 2 * BPC], f32, tag="em")  # col i*2+kc
            for i in range(BPC):
                for c2 in range(2):
                    col = i * 2 + c2
                    nc.sync.dma_start(
                        wv_f32[:, col:col + 1], wv_d[i, c2 * 128:(c2 + 1) * 128, :])
                    nc.sync.dma_start(
                        em_sb[:, col:col + 1], em_d[i, c2 * 128:(c2 + 1) * 128, :])
            if t_dt != f32:
                wv_sb = constp.tile([128, 2 * BPC], t_dt, tag="wvc")
                nc.vector.tensor_copy(wv_sb[:], wv_f32[:])
            else:
                wv_sb = wv_f32
            ones_sb = constp.tile([128, 1], f32, tag="ones")
            nc.sync.dma_start(ones_sb[:], ones_d[:])

            sps_l, vals_l = [], []
            for i in range(BPC):
                # ---- load batch inputs ----
                qT_sb = inp.tile([128, 2 * NQ], f32, tag="qT")  # [:, dt*128+q]
                for dt in range(2):
                    nc.sync.dma_start(
                        qT_sb[:, dt * NQ:(dt + 1) * NQ],
                        qT_d[i, dt * 128:(dt + 1) * 128, :])
                kT_sb = inp.tile([128, 2 * NK], f32, tag="kT")  # [:, dt*256+k]
                for dt in range(2):
                    nc.sync.dma_start(
                        kT_sb[:, dt * NK:(dt + 1) * NK],
                        kT_d[i, dt * 128:(dt + 1) * 128, :])
                v_sb = inp.tile([128, 2 * V], f32, tag="vals")  # [:, kc*256+v]
                for kc in range(2):
                    nc.sync.dma_start(
                        v_sb[:, kc * V:(kc + 1) * V],
                        vals_d[i, kc * 128:(kc + 1) * 128, :])
                vals_l.append(v_sb)

                # ---- projections: q_projT[h,q], k_projT[h,k] ----
                qp_ps = psproj.tile([128, 2 * NQ], f32, tag="qp")
                for hc in range(2):
                    for dt in range(2):
                        nc.tensor.matmul(
                            qp_ps[:, hc * NQ:(hc + 1) * NQ],
                            Wq_sb[:, dt * H + hc * 128: dt * H + hc * 128 + 128],
                            qT_sb[:, dt * NQ:(dt + 1) * NQ],
                            start=(dt == 0), stop=(dt == 1))
                qp_sb = projp.tile([128, 2 * NQ], f32, tag="qp_sb")  # [:, hc*128+q]
                nc.vector.tensor_copy(qp_sb[:], qp_ps[:])
                kp_ps = psproj.tile([128, 2 * NK], f32, tag="kp")
                for hc in range(2):
                    for dt in range(2):
                        nc.tensor.matmul(
                            kp_ps[:, hc * NK:(hc + 1) * NK],
                            Wk_sb[:, dt * H + hc * 128: dt * H + hc * 128 + 128],
                            kT_sb[:, dt * NK:(dt + 1) * NK],
                            start=(dt == 0), stop=(dt == 1))
                kp_sb = projp.tile([128, 2 * NK], f32, tag="kp_sb")  # [:, hc*256+k]
                nc.vector.tensor_copy(kp_sb[:], kp_ps[:])

                # ---- feature chunks: add -> tanh -> weighted reduce ----
                sps = pss.tile([128, 2 * NQ], f32, tag="sps")  # [:, kc*128+q]
                sps_l.append(sps)
                for c in range(NCHUNK):
                    F = featp.tile([128, FW], f32, tag="F")
                    for ql in range(NQC):
                        q = c * NQC + ql
                        for hc in range(2):
                            off = (ql * 2 + hc) * 256
                            nc.vector.tensor_scalar_add(
                                F[:, off:off + 256],
                                kp_sb[:, hc * NK:(hc + 1) * NK],
                                qp_sb[:, hc * 128 + q: hc * 128 + q + 1])
                    T = tanhp.tile([128, FW], t_dt, tag="T")
                    nc.scalar.activation(T[:], F[:], AF.Tanh)
                    for ql in range(NQC):
                        q = c * NQC + ql
                        for kc in range(2):
                            for hc in range(2):
                                off = (ql * 2 + hc) * 256 + kc * 128
                                nc.tensor.matmul(
                                    sps[:, kc * 128 + q: kc * 128 + q + 1],
                                    mm_ap(T[:, off:off + 128]),
                                    mm_ap(wv_sb[:, i * 2 + hc: i * 2 + hc + 1]),
                                    start=(hc == 0), stop=(hc == 1))

            # ---- epilogue: exp, out matmuls, normalize ----
            for i in range(BPC):
                ex = epsp.tile([128, 2 * NQ], f32, tag="ex")  # (k x q) per kc
                for kc in range(2):
                    nc.scalar.activation(
                        ex[:, kc * 128:(kc + 1) * 128],
                        sps_l[i][:, kc * 128:(kc + 1) * 128],
                        AF.Exp, bias=em_sb[:, i * 2 + kc: i * 2 + kc + 1])
                od = pso.tile([128, V + 1], f32, tag="od")  # cols 0:V out, V den
                for kc in range(2):
                    nc.tensor.matmul(
                        od[:, 0:V], ex[:, kc * 128:(kc + 1) * 128],
                        vals_l[i][:, kc * V:(kc + 1) * V],
                        start=(kc == 0), stop=(kc == 1))
                for kc in range(2):
                    nc.tensor.matmul(
                        od[:, V:V + 1], ex[:, kc * 128:(kc + 1) * 128],
                        ones_sb[:], start=(kc == 0), stop=(kc == 1))
                rd = outbp.tile([128, 1], f32, tag="rd")
                nc.vector.reciprocal(rd[:], od[:, V:V + 1])
                o_sb = outbp.tile([128, V], f32, tag="o")
                nc.vector.tensor_scalar_mul(o_sb[:], od[:, 0:V], rd[:])
                nc.sync.dma_start(out_d[i], o_sb[:])

    nc.compile()
    return nc


def get_nc(reps=1, mm_dtype="f32"):
    key = ("nc", reps, mm_dtype)
    if key not in _CACHE:
        _CACHE[key] = _build_nc(reps, mm_dtype)
    return _CACHE[key]


def make_in_maps(queries, keys, values, valid_lens, W_q, W_k, w_v):
    queries = np.asarray(queries, np.float32)
    keys = np.asarray(keys, np.float32)
    values = np.asarray(values, np.float32)
    valid_lens = np.asarray(valid_lens)
    W_q = np.asarray(W_q, np.float32)
    W_k = np.asarray(W_k, np.float32)
    w_v = np.asarray(w_v, np.float32)

    WqT_h = np.ascontiguousarray(W_q.T)
    WkT_h = np.ascontiguousarray(W_k.T)
    ones_h = np.ones((128, 1), np.float32)

    in_maps = []
    for c in range(NCORES):
        sl = slice(BPC * c, BPC * (c + 1))
        qT_h = np.ascontiguousarray(queries[sl].transpose(0, 2, 1))
        kT_h = np.ascontiguousarray(keys[sl].transpose(0, 2, 1))
        vals_h = np.ascontiguousarray(values[sl])
        wv_h = np.zeros((BPC, H, 1), np.float32)
        em_h = np.zeros((BPC, NK, 1), np.float32)
        for i in range(BPC):
            vlen = int(valid_lens[BPC * c + i])
            if vlen > 0:
                wv_h[i, :, 0] = w_v
                em_h[i, vlen:, 0] = -1e6
            # vlen==0: w_v and mask zero -> scores 0 -> uniform softmax
        in_maps.append({
            "qT": qT_h, "kT": kT_h, "vals": vals_h,
            "WqT": WqT_h, "WkT": WkT_h,
            "wv": wv_h, "emask": em_h, "ones": ones_h,
        })
    return in_maps


def kernel(queries, keys, values, valid_lens, W_q, W_k, w_v):
    from concourse.bass_utils import run_bass_kernel_spmd

    nc = get_nc()
    in_maps = make_in_maps(queries, keys, values, valid_lens, W_q, W_k, w_v)
    res = run_bass_kernel_spmd(nc, in_maps, core_ids=list(range(NCORES)))
    out = np.concatenate([res.results[c]["out"] for c in range(NCORES)], axis=0)
    return np.ascontiguousarray(out.astype(np.float32))


# revision 12
# speedup vs baseline: 1.0942x; 1.0942x over previous
"""AdditiveAttention Trainium2 kernel.

Problem (hardcoded shapes): B=16, Nq=128, Nk=256, D=256, H=256, V=256, f32.
  q = queries @ W_q.T ; k = keys @ W_k.T
  scores[b,q,k] = sum_h w_v[h] * tanh(q[b,q,h] + k[b,k,h])
  masked softmax over k (k >= valid_len -> -1e6), out = attn @ values

Sharding: data-parallel, 2 batches per core across 8 cores.

Per-core device program (per batch):
  - PE: q_projT (h x q), k_projT (h x k) from host-pretransposed inputs
  - DVE: feature[h, (q,hc,k)] = k_projT + q_projT[:,q] (per-q tensor_scalar add)
  - ACT: tanh over 8192-wide chunks
  - PE: scoresT[k,q] = sum_h w_v[h]*T via stationary-T matmuls (moving = w_v, N=1)
  - ACT: exp(scoresT + mask_bias)  (no max subtraction: |scores| <= ||w_v||_1)
  - PE: out_unnorm = expT.T @ values, den = expT.T @ ones ; DVE: out = out_unnorm/den
valid_len==0 batches: host zeroes w_v and mask -> scores=0 -> exact uniform softmax,
matching jax softmax of an all -1e6 row.
"""

import numpy as np

B, NQ, NK, D, H, V = 16, 128, 256, 256, 256, 256
NCORES = 8
BPC = B // NCORES  # batches per core
NQC = 16           # q's per feature chunk
NCHUNK = NQ // NQC

_CACHE = {}


def _build_nc(reps=1, mm_dtype="f32"):
    import contextlib
    import concourse.bass as bass
    import concourse.tile as tile
    from concourse import bacc, mybir

    f32 = mybir.dt.float32
    AF = mybir.ActivationFunctionType
    # mm_dtype: "f32" | "bf16" | "f16" (T/w_v in scores matmul) |
    #           "f16all" (also feature adds + projections in fp16 -> DVE 4x)
    t_dt = {"f32": f32, "f32r": f32, "bf16": mybir.dt.bfloat16,
            "f16": mybir.dt.float16, "f16all": mybir.dt.float16}[mm_dtype]
    feat_dt = mybir.dt.float16 if mm_dtype == "f16all" else f32

    def mm_ap(ap):
        return ap.bitcast(mybir.dt.float32r) if mm_dtype == "f32r" else ap

    nc = bacc.Bacc("TRN2")
    qT_d = nc.dram_tensor("qT", (BPC, D, NQ), f32, kind="ExternalInput")
    kT_d = nc.dram_tensor("kT", (BPC, D, NK), f32, kind="ExternalInput")
    vals_d = nc.dram_tensor("vals", (BPC, NK, V), f32, kind="ExternalInput")
    WqT_d = nc.dram_tensor("WqT", (D, H), f32, kind="ExternalInput")
    WkT_d = nc.dram_tensor("WkT", (D, H), f32, kind="ExternalInput")
    wv_d = nc.dram_tensor("wv", (BPC, H, 1), f32, kind="ExternalInput")
    em_d = nc.dram_tensor("emask", (BPC, NK, 1), f32, kind="ExternalInput")
    ones_d = nc.dram_tensor("ones", (128, 1), f32, kind="ExternalInput")
    out_d = nc.dram_tensor("out", (BPC, NQ, V), f32, kind="ExternalOutput")

    FW = 2 * NQC * 256  # feature chunk free width (q-local, hc, k)

    with tile.TileContext(nc) as tc:
        rep_loop = tc.For_i(0, reps, 1) if reps != 1 else contextlib.nullcontext()
        with (
            rep_loop,
            tc.tile_pool(name="const", bufs=1) as constp,
            tc.tile_pool(name="inb", bufs=2) as inp,
            tc.tile_pool(name="proj", bufs=2) as projp,
            tc.tile_pool(name="feat", bufs=2) as featp,
            tc.tile_pool(name="tanh", bufs=2) as tanhp,
            tc.tile_pool(name="eps", bufs=2) as epsp,
            tc.tile_pool(name="outb", bufs=2) as outbp,
            tc.tile_pool(name="ps_proj", bufs=1, space=bass.MemorySpace.PSUM) as psproj,
            tc.tile_pool(name="ps_s", bufs=2, space=bass.MemorySpace.PSUM) as pss,
            tc.tile_pool(name="ps_o", bufs=2, space=bass.MemorySpace.PSUM) as pso,
        ):
            # ---- constants ----
            Wq_sb = constp.tile([128, 2 * H], f32, tag="Wq")  # [:, dt*256+h]
            Wk_sb = constp.tile([128, 2 * H], f32, tag="Wk")
            for dt in range(2):
                nc.sync.dma_start(
                    Wq_sb[:, dt * H:(dt + 1) * H], WqT_d[dt * 128:(dt + 1) * 128, :])
                nc.sync.dma_start(
                    Wk_sb[:, dt * H:(dt + 1) * H], WkT_d[dt * 128:(dt + 1) * 128, :])
            wv_f32 = constp.tile([128, 2 * BPC], f32, tag="wvf")  # col i*2+hc
            em_sb = constp.tile([128, 2 * BPC], f32, tag="em")  # col i*2+kc
            for i in range(BPC):
                for c2 in range(2):
                    col = i * 2 + c2
                    nc.sync.dma_start(
                        wv_f32[:, col:col + 1], wv_d[i, c2 * 128:(c2 + 1) * 128, :])
                    nc.sync.dma_start(
                        em_sb[:, col:col + 1], em_d[i, c2 * 128:(c2 + 1) * 128, :])
            if t_dt != f32:
                wv_sb = constp.tile([128, 2 * BPC], t_dt, tag="wvc")
                nc.vector.tensor_copy(wv_sb[:], wv_f32[:])
            else:
                wv_sb = wv_f32
            ones_sb = constp.tile([128, 1], f32, tag="ones")
            nc.sync.dma_start(ones_sb[:], ones_d[:])

            sps_l, vals_l = [], []
            for i in range(BPC):
                # ---- load batch inputs ----
                qT_sb = inp.tile([128, 2 * NQ], f32, tag="qT")  # [:, dt*128+q]
                for dt in range(2):
                    nc.sync.dma_start(
                        qT_sb[:, dt * NQ:(dt + 1) * NQ],
                        qT_d[i, dt * 128:(dt + 1) * 128, :])
                kT_sb = inp.tile([128, 2 * NK], f32, tag="kT")  # [:, dt*256+k]
                for dt in range(2):
                    nc.sync.dma_start(
                        kT_sb[:, dt * NK:(dt + 1) * NK],
                        kT_d[i, dt * 128:(dt + 1) * 128, :])
                v_sb = inp.tile([128, 2 * V], f32, tag="vals")  # [:, kc*256+v]
                for kc in range(2):
                    nc.sync.dma_start(
                        v_sb[:, kc * V:(kc + 1) * V],
                        vals_d[i, kc * 128:(kc + 1) * 128, :])
                vals_l.append(v_sb)

                # ---- projections: q_projT[h,q], k_projT[h,k] ----
                qp_ps = psproj.tile([128, 2 * NQ], f32, tag="qp")
                for hc in range(2):
                    for dt in range(2):
                        nc.tensor.matmul(
                            qp_ps[:, hc * NQ:(hc + 1) * NQ],
                            Wq_sb[:, dt * H + hc * 128: dt * H + hc * 128 + 128],
                            qT_sb[:, dt * NQ:(dt + 1) * NQ],
                            start=(dt == 0), stop=(dt == 1))
                qp_sb = projp.tile([128, 2 * NQ], feat_dt, tag="qp_sb")
                nc.vector.tensor_copy(qp_sb[:], qp_ps[:])
                kp_ps = psproj.tile([128, 2 * NK], f32, tag="kp")
                for hc in range(2):
                    for dt in range(2):
                        nc.tensor.matmul(
                            kp_ps[:, hc * NK:(hc + 1) * NK],
                            Wk_sb[:, dt * H + hc * 128: dt * H + hc * 128 + 128],
                            kT_sb[:, dt * NK:(dt + 1) * NK],
                            start=(dt == 0), stop=(dt == 1))
                kp_sb = projp.tile([128, 2 * NK], feat_dt, tag="kp_sb")
                nc.vector.tensor_copy(kp_sb[:], kp_ps[:])

                # ---- feature chunks: add -> tanh -> weighted reduce ----
                sps = pss.tile([128, 2 * NQ], f32, tag="sps")  # [:, kc*128+q]
                sps_l.append(sps)
                for c in range(NCHUNK):
                    F = featp.tile([128, FW], feat_dt, tag="F")
                    for ql in range(NQC):
                        q = c * NQC + ql
                        for hc in range(2):
                            off = (ql * 2 + hc) * 256
                            nc.vector.tensor_scalar_add(
                                F[:, off:off + 256],
                                kp_sb[:, hc * NK:(hc + 1) * NK],
                                qp_sb[:, hc * 128 + q: hc * 128 + q + 1])
                    T = tanhp.tile([128, FW], t_dt, tag="T")
                    nc.scalar.activation(T[:], F[:], AF.Tanh)
                    for ql in range(NQC):
                        q = c * NQC + ql
                        for kc in range(2):
                            for hc in range(2):
                                off = (ql * 2 + hc) * 256 + kc * 128
                                nc.tensor.matmul(
                                    sps[:, kc * 128 + q: kc * 128 + q + 1],
                                    mm_ap(T[:, off:off + 128]),
                                    mm_ap(wv_sb[:, i * 2 + hc: i * 2 + hc + 1]),
                                    start=(hc == 0), stop=(hc == 1))

            # ---- epilogue: exp, out matmuls, normalize ----
            for i in range(BPC):
                ex = epsp.tile([128, 2 * NQ], f32, tag="ex")  # (k x q) per kc
                for kc in range(2):
                    nc.scalar.activation(
                        ex[:, kc * 128:(kc + 1) * 128],
                        sps_l[i][:, kc * 128:(kc + 1) * 128],
                        AF.Exp, bias=em_sb[:, i * 2 + kc: i * 2 + kc + 1])
                od = pso.tile([128, V + 1], f32, tag="od")  # cols 0:V out, V den
                for kc in range(2):
                    nc.tensor.matmul(
                        od[:, 0:V], ex[:, kc * 128:(kc + 1) * 128],
                        vals_l[i][:, kc * V:(kc + 1) * V],
                        start=(kc == 0), stop=(kc == 1))
                for kc in range(2):
                    nc.tensor.matmul(
                        od[:, V:V + 1], ex[:, kc * 128:(kc + 1) * 128],
                        ones_sb[:], start=(kc == 0), stop=(kc == 1))
                rd = outbp.tile([128, 1], f32, tag="rd")
                nc.vector.reciprocal(rd[:], od[:, V:V + 1])
                o_sb = outbp.tile([128, V], f32, tag="o")
                nc.vector.tensor_scalar_mul(o_sb[:], od[:, 0:V], rd[:])
                nc.sync.dma_start(out_d[i], o_sb[:])

    nc.compile()
    return nc


def get_nc(reps=1, mm_dtype="f32"):
    key = ("nc", reps, mm_dtype)
    if key not in _CACHE:
        _CACHE[key] = _build_nc(reps, mm_dtype)
    return _CACHE[key]


def make_in_maps(queries, keys, values, valid_lens, W_q, W_k, w_v):
    queries = np.asarray(queries, np.float32)
    keys = np.asarray(keys, np.float32)
    values = np.asarray(values, np.float32)
    valid_lens = np.asarray(valid_lens)
    W_q = np.asarray(W_q, np.float32)
    W_k = np.asarray(W_k, np.float32)
    w_v = np.asarray(w_v, np.float32)

    WqT_h = np.ascontiguousarray(W_q.T)
    WkT_h = np.ascontiguousarray(W_k.T)
    ones_h = np.ones((128, 1), np.float32)

    in_maps = []
    for c in range(NCORES):
        sl = slice(BPC * c, BPC * (c + 1))
        qT_h = np.ascontiguousarray(queries[sl].transpose(0, 2, 1))
        kT_h = np.ascontiguousarray(keys[sl].transpose(0, 2, 1))
        vals_h = np.ascontiguousarray(values[sl])
        wv_h = np.zeros((BPC, H, 1), np.float32)
        em_h = np.zeros((BPC, NK, 1), np.float32)
        for i in range(BPC):
            vlen = int(valid_lens[BPC * c + i])
            if vlen > 0:
                wv_h[i, :, 0] = w_v
                em_h[i, vlen:, 0] = -1e6
            # vlen==0: w_v and mask zero -> scores 0 -> uniform softmax
        in_maps.append({
            "qT": qT_h, "kT": kT_h, "vals": vals_h,
            "WqT": WqT_h, "WkT": WkT_h,
            "wv": wv_h, "emask": em_h, "ones": ones_h,
        })
    return in_maps


def kernel(queries, keys, values, valid_lens, W_q, W_k, w_v):
    from concourse.bass_utils import run_bass_kernel_spmd

    nc = get_nc()
    in_maps = make_in_maps(queries, keys, values, valid_lens, W_q, W_k, w_v)
    res = run_bass_kernel_spmd(nc, in_maps, core_ids=list(range(NCORES)))
    out = np.concatenate([res.results[c]["out"] for c in range(NCORES)], axis=0)
    return np.ascontiguousarray(out.astype(np.float32))


# revision 13
# speedup vs baseline: 1.2640x; 1.1551x over previous
"""AdditiveAttention Trainium2 kernel.

Problem (hardcoded shapes): B=16, Nq=128, Nk=256, D=256, H=256, V=256, f32.
  q = queries @ W_q.T ; k = keys @ W_k.T
  scores[b,q,k] = sum_h w_v[h] * tanh(q[b,q,h] + k[b,k,h])
  masked softmax over k (k >= valid_len -> -1e6), out = attn @ values

Sharding: data-parallel, 2 batches per core across 8 cores.

Per-core device program (per batch):
  - PE: q_projT (h x q), k_projT (h x k) from host-pretransposed inputs
  - DVE: feature[h, (q,hc,k)] = k_projT + q_projT[:,q] (per-q tensor_scalar add)
  - ACT: tanh over 8192-wide chunks
  - PE: scoresT[k,q] = sum_h w_v[h]*T via stationary-T matmuls (moving = w_v, N=1)
  - ACT: exp(scoresT + mask_bias)  (no max subtraction: |scores| <= ||w_v||_1)
  - PE: out_unnorm = expT.T @ values, den = expT.T @ ones ; DVE: out = out_unnorm/den
valid_len==0 batches: host zeroes w_v and mask -> scores=0 -> exact uniform softmax,
matching jax softmax of an all -1e6 row.
"""

import numpy as np

B, NQ, NK, D, H, V = 16, 128, 256, 256, 256, 256
NCORES = 8
BPC = B // NCORES  # batches per core
NQC = 16           # q's per feature chunk
NCHUNK = NQ // NQC

_CACHE = {}


def _build_nc(reps=1, mm_dtype="f32"):
    import contextlib
    import concourse.bass as bass
    import concourse.tile as tile
    from concourse import bacc, mybir

    f32 = mybir.dt.float32
    AF = mybir.ActivationFunctionType
    # mm_dtype: "f32" | "bf16" | "f16" (T/w_v in scores matmul) |
    #           "f16all" (also feature adds + projections in fp16 -> DVE 4x)
    t_dt = {"f32": f32, "f32r": f32, "bf16": mybir.dt.bfloat16,
            "f16": mybir.dt.float16, "f16all": mybir.dt.float16}[mm_dtype]
    feat_dt = mybir.dt.float16 if mm_dtype == "f16all" else f32

    def mm_ap(ap):
        return ap.bitcast(mybir.dt.float32r) if mm_dtype == "f32r" else ap

    nc = bacc.Bacc("TRN2")
    qT_d = nc.dram_tensor("qT", (BPC, D, NQ), f32, kind="ExternalInput")
    kT_d = nc.dram_tensor("kT", (BPC, D, NK), f32, kind="ExternalInput")
    vals_d = nc.dram_tensor("vals", (BPC, NK, V), f32, kind="ExternalInput")
    WqT_d = nc.dram_tensor("WqT", (D, H), f32, kind="ExternalInput")
    WkT_d = nc.dram_tensor("WkT", (D, H), f32, kind="ExternalInput")
    wv_d = nc.dram_tensor("wv", (BPC, H, 1), f32, kind="ExternalInput")
    em_d = nc.dram_tensor("emask", (BPC, NK, 1), f32, kind="ExternalInput")
    ones_d = nc.dram_tensor("ones", (128, 1), f32, kind="ExternalInput")
    out_d = nc.dram_tensor("out", (BPC, NQ, V), f32, kind="ExternalOutput")

    FW = 2 * NQC * 256  # feature chunk free width (q-local, hc, k)

    with tile.TileContext(nc) as tc:
        rep_loop = tc.For_i(0, reps, 1) if reps != 1 else contextlib.nullcontext()
        with (
            rep_loop,
            tc.tile_pool(name="const", bufs=1) as constp,
            tc.tile_pool(name="inb", bufs=2) as inp,
            tc.tile_pool(name="proj", bufs=2) as projp,
            tc.tile_pool(name="feat", bufs=2) as featp,
            tc.tile_pool(name="tanh", bufs=2) as tanhp,
            tc.tile_pool(name="eps", bufs=2) as epsp,
            tc.tile_pool(name="outb", bufs=2) as outbp,
            tc.tile_pool(name="ps_proj", bufs=1, space=bass.MemorySpace.PSUM) as psproj,
            tc.tile_pool(name="ps_s", bufs=2, space=bass.MemorySpace.PSUM) as pss,
            tc.tile_pool(name="ps_o", bufs=2, space=bass.MemorySpace.PSUM) as pso,
        ):
            # ---- constants ----
            Wq_sb = constp.tile([128, 2 * H], f32, tag="Wq")  # [:, dt*256+h]
            Wk_sb = constp.tile([128, 2 * H], f32, tag="Wk")
            for dt in range(2):
                nc.sync.dma_start(
                    Wq_sb[:, dt * H:(dt + 1) * H], WqT_d[dt * 128:(dt + 1) * 128, :])
                nc.sync.dma_start(
                    Wk_sb[:, dt * H:(dt + 1) * H], WkT_d[dt * 128:(dt + 1) * 128, :])
            wv_f32 = constp.tile([128, 2 * BPC], f32, tag="wvf")  # col i*2+hc
            em_sb = constp.tile([128, 2 * BPC], f32, tag="em")  # col i*2+kc
            for i in range(BPC):
                for c2 in range(2):
                    col = i * 2 + c2
                    nc.sync.dma_start(
                        wv_f32[:, col:col + 1], wv_d[i, c2 * 128:(c2 + 1) * 128, :])
                    nc.sync.dma_start(
                        em_sb[:, col:col + 1], em_d[i, c2 * 128:(c2 + 1) * 128, :])
            if t_dt != f32:
                wv_sb = constp.tile([128, 2 * BPC], t_dt, tag="wvc")
                nc.vector.tensor_copy(wv_sb[:], wv_f32[:])
            else:
                wv_sb = wv_f32
            ones_sb = constp.tile([128, 1], f32, tag="ones")
            nc.sync.dma_start(ones_sb[:], ones_d[:])

            sps_l, vals_l = [], []
            for i in range(BPC):
                # ---- load batch inputs ----
                qT_sb = inp.tile([128, 2 * NQ], f32, tag="qT")  # [:, dt*128+q]
                for dt in range(2):
                    nc.sync.dma_start(
                        qT_sb[:, dt * NQ:(dt + 1) * NQ],
                        qT_d[i, dt * 128:(dt + 1) * 128, :])
                kT_sb = inp.tile([128, 2 * NK], f32, tag="kT")  # [:, dt*256+k]
                for dt in range(2):
                    nc.sync.dma_start(
                        kT_sb[:, dt * NK:(dt + 1) * NK],
                        kT_d[i, dt * 128:(dt + 1) * 128, :])
                v_sb = inp.tile([128, 2 * V], f32, tag="vals")  # [:, kc*256+v]
                for kc in range(2):
                    nc.sync.dma_start(
                        v_sb[:, kc * V:(kc + 1) * V],
                        vals_d[i, kc * 128:(kc + 1) * 128, :])
                vals_l.append(v_sb)

                # ---- projections: q_projT[h,q], k_projT[h,k] ----
                qp_ps = psproj.tile([128, 2 * NQ], f32, tag="qp")
                for hc in range(2):
                    for dt in range(2):
                        nc.tensor.matmul(
                            qp_ps[:, hc * NQ:(hc + 1) * NQ],
                            Wq_sb[:, dt * H + hc * 128: dt * H + hc * 128 + 128],
                            qT_sb[:, dt * NQ:(dt + 1) * NQ],
                            start=(dt == 0), stop=(dt == 1))
                qp_sb = projp.tile([128, 2 * NQ], f32, tag="qp_sb")
                nc.vector.tensor_copy(qp_sb[:], qp_ps[:])
                kp_ps = psproj.tile([128, 2 * NK], f32, tag="kp")
                for hc in range(2):
                    for dt in range(2):
                        nc.tensor.matmul(
                            kp_ps[:, hc * NK:(hc + 1) * NK],
                            Wk_sb[:, dt * H + hc * 128: dt * H + hc * 128 + 128],
                            kT_sb[:, dt * NK:(dt + 1) * NK],
                            start=(dt == 0), stop=(dt == 1))
                kp_sb = projp.tile([128, 2 * NK], feat_dt, tag="kp_sb")
                nc.vector.tensor_copy(kp_sb[:], kp_ps[:])

                # ---- feature chunks: add -> tanh -> weighted reduce ----
                sps = pss.tile([128, 2 * NQ], f32, tag="sps")  # [:, kc*128+q]
                sps_l.append(sps)
                for c in range(NCHUNK):
                    F = featp.tile([128, FW], feat_dt, tag="F")
                    for ql in range(NQC):
                        q = c * NQC + ql
                        for hc in range(2):
                            off = (ql * 2 + hc) * 256
                            nc.vector.tensor_scalar_add(
                                F[:, off:off + 256],
                                kp_sb[:, hc * NK:(hc + 1) * NK],
                                qp_sb[:, hc * 128 + q: hc * 128 + q + 1])
                    T = tanhp.tile([128, FW], t_dt, tag="T")
                    nc.scalar.activation(T[:], F[:], AF.Tanh)
                    for ql in range(NQC):
                        q = c * NQC + ql
                        for kc in range(2):
                            for hc in range(2):
                                off = (ql * 2 + hc) * 256 + kc * 128
                                nc.tensor.matmul(
                                    sps[:, kc * 128 + q: kc * 128 + q + 1],
                                    mm_ap(T[:, off:off + 128]),
                                    mm_ap(wv_sb[:, i * 2 + hc: i * 2 + hc + 1]),
                                    start=(hc == 0), stop=(hc == 1))

            # ---- epilogue: exp, out matmuls, normalize ----
            for i in range(BPC):
                ex = epsp.tile([128, 2 * NQ], f32, tag="ex")  # (k x q) per kc
                for kc in range(2):
                    nc.scalar.activation(
                        ex[:, kc * 128:(kc + 1) * 128],
                        sps_l[i][:, kc * 128:(kc + 1) * 128],
                        AF.Exp, bias=em_sb[:, i * 2 + kc: i * 2 + kc + 1])
                od = pso.tile([128, V + 1], f32, tag="od")  # cols 0:V out, V den
                for kc in range(2):
                    nc.tensor.matmul(
                        od[:, 0:V], ex[:, kc * 128:(kc + 1) * 128],
                        vals_l[i][:, kc * V:(kc + 1) * V],
                        start=(kc == 0), stop=(kc == 1))
                for kc in range(2):
                    nc.tensor.matmul(
                        od[:, V:V + 1], ex[:, kc * 128:(kc + 1) * 128],
                        ones_sb[:], start=(kc == 0), stop=(kc == 1))
                rd = outbp.tile([128, 1], f32, tag="rd")
                nc.vector.reciprocal(rd[:], od[:, V:V + 1])
                o_sb = outbp.tile([128, V], f32, tag="o")
                nc.vector.tensor_scalar_mul(o_sb[:], od[:, 0:V], rd[:])
                nc.sync.dma_start(out_d[i], o_sb[:])

    nc.compile()
    return nc


def get_nc(reps=1, mm_dtype="f32"):
    key = ("nc", reps, mm_dtype)
    if key not in _CACHE:
        _CACHE[key] = _build_nc(reps, mm_dtype)
    return _CACHE[key]


def make_in_maps(queries, keys, values, valid_lens, W_q, W_k, w_v):
    queries = np.asarray(queries, np.float32)
    keys = np.asarray(keys, np.float32)
    values = np.asarray(values, np.float32)
    valid_lens = np.asarray(valid_lens)
    W_q = np.asarray(W_q, np.float32)
    W_k = np.asarray(W_k, np.float32)
    w_v = np.asarray(w_v, np.float32)

    WqT_h = np.ascontiguousarray(W_q.T)
    WkT_h = np.ascontiguousarray(W_k.T)
    ones_h = np.ones((128, 1), np.float32)

    in_maps = []
    for c in range(NCORES):
        sl = slice(BPC * c, BPC * (c + 1))
        qT_h = np.ascontiguousarray(queries[sl].transpose(0, 2, 1))
        kT_h = np.ascontiguousarray(keys[sl].transpose(0, 2, 1))
        vals_h = np.ascontiguousarray(values[sl])
        wv_h = np.zeros((BPC, H, 1), np.float32)
        em_h = np.zeros((BPC, NK, 1), np.float32)
        for i in range(BPC):
            vlen = int(valid_lens[BPC * c + i])
            if vlen > 0:
                wv_h[i, :, 0] = w_v
                em_h[i, vlen:, 0] = -1e6
            # vlen==0: w_v and mask zero -> scores 0 -> uniform softmax
        in_maps.append({
            "qT": qT_h, "kT": kT_h, "vals": vals_h,
            "WqT": WqT_h, "WkT": WkT_h,
            "wv": wv_h, "emask": em_h, "ones": ones_h,
        })
    return in_maps


def kernel(queries, keys, values, valid_lens, W_q, W_k, w_v):
    from concourse.bass_utils import run_bass_kernel_spmd

    nc = get_nc()
    in_maps = make_in_maps(queries, keys, values, valid_lens, W_q, W_k, w_v)
    res = run_bass_kernel_spmd(nc, in_maps, core_ids=list(range(NCORES)))
    out = np.concatenate([res.results[c]["out"] for c in range(NCORES)], axis=0)
    return np.ascontiguousarray(out.astype(np.float32))
